# revision 12
# baseline (speedup 1.0000x reference)
"""Luong 'general' attention decode step on 8 TRN2 NeuronCores.

Math (per batch b):
    q[b]      = state[b] @ Wa                      # [H]   (reassociated projection)
    score[s]  = enc[b,s,:] . q[b]                  # [S]
    w         = softmax(score)                     # [S]
    ctx[b]    = sum_s w[s] * enc[b,s,:]            # [H]
    out[b]    = relu(tanh([ctx, state]) @ Wc^T + bias)

Sharding: data-parallel over batch (32 batches -> 4 per core). Weights
replicated. No collectives.

Per-core pipeline (all fp32 except the final Wc matmul):
    - DMA enc in 1MB slabs straight into a per-batch fp32 SBUF store.
    - Scores: DVE tensor_tensor multiply (enc * q_broadcast), then ACT
      activation(Copy, accum_out=...) reduces along h - splitting the dot
      product across the two engines so each does one pass per tile.
    - Softmax: per-partition max/exp/sum, cross-partition combine via
      TensorE transposes, fold exp(m_p - M)/Z into a per-partition scale.
    - Context: TensorE with the enc chunk as the fp32 stationary operand and
      the weight column moving: ctxT[:, hc] += enc[:, t, hc]^T @ w[:, t].
      Output lands h-on-partitions, exactly the layout the final matmul
      needs - no epilogue transposes.
    - Epilogue: one tanh per batch (PSUM -> bf16), final matmul against
      host-pre-transposed Wc^T in bf16, add bias, relu.
"""

import numpy as np
import ml_dtypes

import concourse.bass as bass
import concourse.tile as tile
import concourse.mybir as mybir
from concourse import bacc
from concourse.bass_utils import run_bass_kernel_spmd

N_CORES = 8
B, S, H = 32, 2048, 1024
BL = B // N_CORES            # batches per core
P = 128                      # partitions
NT = S // P                  # 16 s-tiles per batch
TPS = 2                      # tiles per DMA slab (1MB slabs)
NSLAB = NT // TPS
HC = H // P                  # 8 h-chunks
JC = 2 * H // P              # 16 j-chunks of the concat axis

F32 = mybir.dt.float32
BF16 = mybir.dt.bfloat16
AF = mybir.ActivationFunctionType
ALU = mybir.AluOpType
AX = mybir.AxisListType

_cache = {}


def _body(tc, enc, stateT, wa, wct, bias, ident, out_state, out_attn):
    nc = tc.nc
    with (
        tc.tile_pool(name="singles", bufs=1) as singles,
        tc.tile_pool(name="encst", bufs=2) as encst_pool,
        tc.tile_pool(name="scr", bufs=2) as scr_pool,
        tc.tile_pool(name="pb", bufs=2) as pb_pool,
        tc.tile_pool(name="psmm", bufs=4, space=bass.MemorySpace.PSUM) as psmm,
        tc.tile_pool(name="pstp", bufs=2, space=bass.MemorySpace.PSUM) as pstp,
    ):
        # ---------- prologue: constants ----------
        wa_sb = singles.tile([P, HC, H], F32, tag="wgt")
        nc.sync.dma_start(out=wa_sb[:], in_=wa.rearrange("(c p) h -> p c h", p=P))
        ident_sb = singles.tile([P, P], F32)
        nc.gpsimd.dma_start(out=ident_sb[:], in_=ident)
        stT_sb = singles.tile([P, HC, BL], F32)
        nc.gpsimd.dma_start(
            out=stT_sb[:], in_=stateT.rearrange("(c p) b -> p c b", p=P)
        )
        bias_sb = singles.tile([BL, H], F32)
        nc.gpsimd.dma_start(out=bias_sb[:], in_=bias)

        # tanh(state)^T in bf16 (state half of the final concat matmul)
        stT_tanh_bf = singles.tile([P, HC, BL], BF16)
        nc.scalar.activation(stT_tanh_bf[:], stT_sb[:], AF.Tanh)

        # ---------- q = state @ Wa  -> [BL, H] ----------
        q_ps = [
            psmm.tile([BL, 512], F32, tag="mm", name=f"q_ps{i}") for i in range(2)
        ]
        for c in range(HC):
            for nh in range(2):
                nc.tensor.matmul(
                    q_ps[nh][:],
                    stT_sb[:, c, :],
                    wa_sb[:, c, nh * 512 : (nh + 1) * 512],
                    start=(c == 0),
                    stop=(c == HC - 1),
                )
        q_sb = singles.tile([BL, H], F32)
        for nh in range(2):
            nc.scalar.copy(q_sb[:, nh * 512 : (nh + 1) * 512], q_ps[nh][:])

        # Wc^T (bf16) reuses Wa's SBUF slot once q is done (same tag)
        wct_sb = singles.tile([P, JC, H], BF16, tag="wgt")
        nc.gpsimd.dma_start(out=wct_sb[:], in_=wct.rearrange("(c p) o -> p c o", p=P))

        # ---------- broadcast q[b] to all 128 partitions ----------
        # DRAM bounce + partition-stride-0 read replicates q across partitions.
        q_dram = nc.dram_tensor(f"q_dram_{nc.next_id()}", [BL, H], F32).ap()
        nc.gpsimd.dma_start(out=q_dram, in_=q_sb[:])
        q_bc = singles.tile([P, BL, H], F32)
        q_rep = bass.AP(
            tensor=q_dram.tensor,
            offset=q_dram.offset,
            ap=[[0, P], q_dram.ap[0], q_dram.ap[1]],
        )
        nc.gpsimd.dma_start(out=q_bc[:], in_=q_rep)

        # tanh(ctx)^T assembled across batches for the final matmul
        tcT_bf = singles.tile([P, HC, BL], BF16)

        # ---------- streaming main loop ----------
        for b in range(BL):
            scores = pb_pool.tile([P, NT], F32, tag="scores")
            enc_st = encst_pool.tile([P, NT, H], F32, tag="encst")

            for sl in range(NSLAB):
                nc.sync.dma_start(
                    out=enc_st[:, sl * TPS : (sl + 1) * TPS, :],
                    in_=enc[b, sl * TPS * P : (sl + 1) * TPS * P, :].rearrange(
                        "(i p) h -> p i h", p=P
                    ),
                )
                for i in range(TPS):
                    t = sl * TPS + i
                    prod = scr_pool.tile([P, H], F32, tag="prod")
                    nc.vector.tensor_tensor(
                        out=prod[:], in0=enc_st[:, t, :], in1=q_bc[:, b], op=ALU.mult
                    )
                    cp = scr_pool.tile([P, H], F32, tag="cp", bufs=1)
                    nc.scalar.activation(
                        cp[:], prod[:], AF.Copy, accum_out=scores[:, t : t + 1]
                    )

            # ---------- softmax over the full 2048 scores ----------
            negm_col = pb_pool.tile([P, 1], F32, tag="negmcol")
            nc.vector.tensor_reduce(
                negm_col[:], scores[:], axis=AX.X, op=ALU.max, negate=True
            )
            e_sb = pb_pool.tile([P, NT], F32, tag="esb")
            s_col = pb_pool.tile([P, 1], F32, tag="scol")
            nc.scalar.activation(
                e_sb[:], scores[:], AF.Exp, bias=negm_col[:], scale=1.0,
                accum_out=s_col[:],
            )
            # bring the 128 per-partition stats onto one partition
            negm_ps = pstp.tile([1, P], F32, tag="tp")
            nc.tensor.transpose(negm_ps[:], negm_col[:], ident_sb[:])
            negm_row = pb_pool.tile([1, P], F32, tag="negmrow")
            nc.vector.tensor_copy(negm_row[:], negm_ps[:])
            s_ps = pstp.tile([1, P], F32, tag="tp")
            nc.tensor.transpose(s_ps[:], s_col[:], ident_sb[:])
            s_row = pb_pool.tile([1, P], F32, tag="srow")
            nc.vector.tensor_copy(s_row[:], s_ps[:])
            # negM = min over partitions of (-rowmax)  (=> M = global max)
            negM = pb_pool.tile([1, 1], F32, tag="negM")
            nc.vector.tensor_reduce(negM[:], negm_row[:], axis=AX.X, op=ALU.min)
            # expm[p] = exp(rowmax_p - M) = exp(-(-rowmax_p) + negM)
            expm = pb_pool.tile([1, P], F32, tag="expm")
            nc.scalar.activation(
                expm[:], negm_row[:], AF.Exp, bias=negM[:], scale=-1.0
            )
            # Z = sum_p rowsum_p * expm_p
            zrow = pb_pool.tile([1, P], F32, tag="zrow")
            nc.vector.tensor_tensor(
                out=zrow[:], in0=s_row[:], in1=expm[:], op=ALU.mult
            )
            zg = pb_pool.tile([1, 1], F32, tag="zg")
            nc.vector.tensor_reduce(zg[:], zrow[:], axis=AX.X, op=ALU.add)
            rz = pb_pool.tile([1, 1], F32, tag="rz")
            nc.vector.reciprocal(rz[:], zg[:])
            prow = pb_pool.tile([1, P], F32, tag="prow")
            nc.vector.tensor_scalar_mul(prow[:], expm[:], rz[:])
            c_ps = pstp.tile([P, 1], F32, tag="tp")
            nc.tensor.transpose(c_ps[:], prow[:], ident_sb[0:1, 0:1])
            c_col = pb_pool.tile([P, 1], F32, tag="ccol")
            nc.vector.tensor_copy(c_col[:], c_ps[:])

            # normalized softmax weights (fp32)
            w_sb = pb_pool.tile([P, NT], F32, tag="wsb")
            nc.vector.tensor_scalar_mul(w_sb[:], e_sb[:], c_col[:])

            # attn output: transpose w to [t, p] so DRAM write is contiguous
            wT_ps = pstp.tile([NT, P], F32, tag="tp")
            nc.tensor.transpose(wT_ps[:], w_sb[:], ident_sb[:])
            wT_sb = pb_pool.tile([NT, P], F32, tag="wTsb")
            nc.vector.tensor_copy(wT_sb[:], wT_ps[:])
            nc.gpsimd.dma_start(
                out=out_attn[b].rearrange("(t p) -> t p", p=P), in_=wT_sb[:]
            )

            # ---------- context, transposed: ctxT[h] = sum_s enc[s,h] w[s] ----------
            # enc chunk is the fp32 stationary operand; w column is the moving
            # operand. Result lands [h, 1] per chunk - already transposed.
            ctxT_ps = psmm.tile([P, HC], F32, tag="mm")
            for hc in range(HC):
                for t in range(NT):
                    nc.tensor.matmul(
                        ctxT_ps[:, hc : hc + 1],
                        enc_st[:, t, hc * P : (hc + 1) * P],
                        w_sb[:, t : t + 1],
                        start=(t == 0),
                        stop=(t == NT - 1),
                    )
            # tanh straight out of PSUM into the assembled bf16 [j, b] layout
            nc.scalar.activation(tcT_bf[:, :, b], ctxT_ps[:], AF.Tanh)

        # ---------- final: relu(tanh([ctx, state]) @ Wc^T + bias) ----------
        f_ps = [
            psmm.tile([BL, 512], F32, tag="mm", name=f"f_ps{i}") for i in range(2)
        ]
        for jc in range(JC):
            lhsT = tcT_bf[:, jc, :] if jc < HC else stT_tanh_bf[:, jc - HC, :]
            for nh in range(2):
                nc.tensor.matmul(
                    f_ps[nh][:],
                    lhsT,
                    wct_sb[:, jc, nh * 512 : (nh + 1) * 512],
                    start=(jc == 0),
                    stop=(jc == JC - 1),
                )
        fout = singles.tile([BL, H], F32)
        for nh in range(2):
            nc.vector.tensor_tensor(
                out=fout[:, nh * 512 : (nh + 1) * 512],
                in0=f_ps[nh][:],
                in1=bias_sb[:, nh * 512 : (nh + 1) * 512],
                op=ALU.add,
            )
        nc.vector.tensor_scalar_max(fout[:], fout[:], 0.0)
        nc.gpsimd.dma_start(out=out_state, in_=fout[:])


def _build(reps=1):
    nc = bacc.Bacc(
        "TRN2", target_bir_lowering=False, debug=False, num_devices=N_CORES
    )
    enc = nc.dram_tensor("enc", [BL, S, H], F32, kind="ExternalInput").ap()
    stateT = nc.dram_tensor("stateT", [H, BL], F32, kind="ExternalInput").ap()
    wa = nc.dram_tensor("wa", [H, H], F32, kind="ExternalInput").ap()
    wct = nc.dram_tensor("wct", [2 * H, H], BF16, kind="ExternalInput").ap()
    bias = nc.dram_tensor("bias", [BL, H], F32, kind="ExternalInput").ap()
    ident = nc.dram_tensor("ident", [P, P], F32, kind="ExternalInput").ap()
    out_state = nc.dram_tensor("out_state", [BL, H], F32, kind="ExternalOutput").ap()
    out_attn = nc.dram_tensor("out_attn", [BL, S], F32, kind="ExternalOutput").ap()

    with tile.TileContext(nc) as tc:
        for _ in range(reps):
            _body(tc, enc, stateT, wa, wct, bias, ident, out_state, out_attn)
    nc.compile()
    return nc


def get_nc(reps=1):
    key = f"nc{reps}"
    if key not in _cache:
        _cache[key] = _build(reps)
    return _cache[key]


def make_in_maps(encoder_output, target_state, Wa_w, Wc_w, Wc_b):
    enc = np.asarray(encoder_output, dtype=np.float32)
    state = np.asarray(target_state, dtype=np.float32)[0]        # [B, H]
    wa = np.ascontiguousarray(np.asarray(Wa_w, dtype=np.float32))
    wct = np.ascontiguousarray(
        np.asarray(Wc_w, dtype=np.float32).T
    ).astype(ml_dtypes.bfloat16)                                 # [2H, H] bf16
    bias = np.tile(np.asarray(Wc_b, dtype=np.float32), (BL, 1))  # [BL, H]
    ident = np.eye(P, dtype=np.float32)

    in_maps = []
    for i in range(N_CORES):
        bs = slice(i * BL, (i + 1) * BL)
        in_maps.append(
            {
                "enc": np.ascontiguousarray(enc[bs]),
                "stateT": np.ascontiguousarray(state[bs].T),
                "wa": wa,
                "wct": wct,
                "bias": bias,
                "ident": ident,
            }
        )
    return in_maps


def kernel(encoder_output, target_state, Wa_w, Wc_w, Wc_b, **run_kwargs):
    nc = get_nc()
    in_maps = make_in_maps(encoder_output, target_state, Wa_w, Wc_w, Wc_b)
    res = run_bass_kernel_spmd(nc, in_maps, core_ids=list(range(N_CORES)), **run_kwargs)
    outs = res.results
    output = np.concatenate([outs[i]["out_state"] for i in range(N_CORES)], axis=0)
    attn = np.concatenate([outs[i]["out_attn"] for i in range(N_CORES)], axis=0)
    kernel.last_results = res
    return output[None].astype(np.float32), attn.astype(np.float32)


# revision 13
# speedup vs baseline: 21.9466x; 21.9466x over previous
"""Luong 'general' attention decode step on 8 TRN2 NeuronCores.

Math (per batch b):
    q[b]      = state[b] @ Wa                      # [H]   (reassociated projection)
    score[s]  = enc[b,s,:] . q[b]                  # [S]
    w         = softmax(score)                     # [S]
    ctx[b]    = sum_s w[s] * enc[b,s,:]            # [H]
    out[b]    = relu(tanh([ctx, state]) @ Wc^T + bias)

Sharding: data-parallel over batch (32 batches -> 4 per core). Weights
replicated. No collectives.

Per-core pipeline:
    - DMA enc in 1MB slabs into a small fp32 ring.
    - Scores in fp32 (attn output precision needs it): DVE tensor_tensor
      multiply against broadcast q, then a free-axis add-reduce; reduces are
      split DVE/ACT to balance the two engines (the dot product costs two
      elementwise passes per tile and the engines co-own them).
    - Each tile is also cast fp32 -> bf16 (ACT) into a per-batch store that
      feeds the context matmul at full PE rate.
    - Softmax: per-partition max/exp/sum, cross-partition combine via
      TensorE transposes, fold exp(m_p - M)/Z into a per-partition scale.
    - Context: TensorE, w column stationary (bf16), enc_bf moving.
    - Epilogue: tanh, 8 chunk transposes to [j, b] layout, final matmul
      against host-pre-transposed Wc^T in bf16, add bias, relu.
    - The PE HAM clock gate idles to 1.2 GHz after ~3.4us without matmuls:
      warmup matmuls at t=0 and one tiny per-slab keep-warm matmul (reads
      the just-DMA'd slab, so it is naturally paced) hold 2.4 GHz.
"""

import numpy as np
import ml_dtypes

import concourse.bass as bass
import concourse.tile as tile
import concourse.mybir as mybir
from concourse import bacc
from concourse.bass_utils import run_bass_kernel_spmd

N_CORES = 8
B, S, H = 32, 2048, 1024
BL = B // N_CORES            # batches per core
P = 128                      # partitions
NT = S // P                  # 16 s-tiles per batch
TPS = 2                      # tiles per DMA slab (1MB slabs)
NSLAB = NT // TPS
HC = H // P                  # 8 h-chunks
JC = 2 * H // P              # 16 j-chunks of the concat axis

F32 = mybir.dt.float32
BF16 = mybir.dt.bfloat16
AF = mybir.ActivationFunctionType
ALU = mybir.AluOpType
AX = mybir.AxisListType

# of the 16 reduces per batch, this many run on DVE (rest on ACT)
REDUCE_DVE = 7

_cache = {}


def _body(tc, enc, stateT, wa, wct, bias, ident, out_state, out_attn):
    nc = tc.nc
    with (
        tc.tile_pool(name="singles", bufs=1) as singles,
        tc.tile_pool(name="encraw", bufs=4) as encraw_pool,
        tc.tile_pool(name="encbf", bufs=2) as encbf_pool,
        tc.tile_pool(name="scr", bufs=2) as scr_pool,
        tc.tile_pool(name="pb", bufs=2) as pb_pool,
        tc.tile_pool(name="psmm", bufs=4, space=bass.MemorySpace.PSUM) as psmm,
        tc.tile_pool(name="pstp", bufs=2, space=bass.MemorySpace.PSUM) as pstp,
        tc.tile_pool(name="pswarm", bufs=1, space=bass.MemorySpace.PSUM) as pswarm,
    ):
        # ---------- PE warmup: ~4us of matmuls so HAM ungates to 2.4GHz ----------
        warm_sb = singles.tile([P, 512], BF16)
        nc.vector.memset(warm_sb[:], 0.0)
        warm_ps = pswarm.tile([P, 512], F32)
        for i in range(10):
            nc.tensor.matmul(
                warm_ps[:], warm_sb[:, 0:128], warm_sb[:], start=True, stop=True
            )

        # ---------- prologue: constants ----------
        # Wa arrives in 8 chunk DMAs so the q matmuls pipeline with the load.
        wa_sb = singles.tile([P, HC, H], F32, tag="wgt")
        wa_r = wa.rearrange("(c p) h -> p c h", p=P)
        for c in range(HC):
            nc.sync.dma_start(out=wa_sb[:, c], in_=wa_r[:, c])
        ident_sb = singles.tile([P, P], F32)
        nc.gpsimd.dma_start(out=ident_sb[:], in_=ident)
        stT_sb = singles.tile([P, HC, BL], F32)
        nc.gpsimd.dma_start(
            out=stT_sb[:], in_=stateT.rearrange("(c p) b -> p c b", p=P)
        )
        bias_sb = singles.tile([BL, H], F32)
        nc.gpsimd.dma_start(out=bias_sb[:], in_=bias)

        # tanh(state)^T in bf16 (state half of the final concat matmul)
        stT_tanh_bf = singles.tile([P, HC, BL], BF16)
        nc.scalar.activation(stT_tanh_bf[:], stT_sb[:], AF.Tanh)

        # ---------- q = state @ Wa  -> [BL, H] ----------
        q_ps = [
            psmm.tile([BL, 512], F32, tag="mm", name=f"q_ps{i}") for i in range(2)
        ]
        for c in range(HC):
            for nh in range(2):
                nc.tensor.matmul(
                    q_ps[nh][:],
                    stT_sb[:, c, :],
                    wa_sb[:, c, nh * 512 : (nh + 1) * 512],
                    start=(c == 0),
                    stop=(c == HC - 1),
                )
        q_sb = singles.tile([BL, H], F32)
        for nh in range(2):
            nc.scalar.copy(q_sb[:, nh * 512 : (nh + 1) * 512], q_ps[nh][:])

        # Wc^T (bf16) reuses Wa's SBUF slot once q is done (same tag)
        wct_sb = singles.tile([P, JC, H], BF16, tag="wgt")
        nc.gpsimd.dma_start(out=wct_sb[:], in_=wct.rearrange("(c p) o -> p c o", p=P))

        # ---------- broadcast q[b] to all 128 partitions ----------
        # DRAM bounce + partition-stride-0 read replicates q across partitions.
        q_dram = nc.dram_tensor(f"q_dram_{nc.next_id()}", [BL, H], F32).ap()
        nc.gpsimd.dma_start(out=q_dram, in_=q_sb[:])
        q_bc = singles.tile([P, BL, H], F32)
        q_rep = bass.AP(
            tensor=q_dram.tensor,
            offset=q_dram.offset,
            ap=[[0, P], q_dram.ap[0], q_dram.ap[1]],
        )
        nc.gpsimd.dma_start(out=q_bc[:], in_=q_rep)

        # tanh(ctx)^T assembled across batches for the final matmul
        tcT_bf = singles.tile([P, HC, BL], BF16)

        # ---------- streaming main loop ----------
        for b in range(BL):
            scores = pb_pool.tile([P, NT], F32, tag="scores")
            enc_bf = encbf_pool.tile([P, NT, H], BF16, tag="encbf")

            for sl in range(NSLAB):
                raw = encraw_pool.tile([P, TPS, H], F32, tag="raw")
                nc.sync.dma_start(
                    out=raw[:],
                    in_=enc[b, sl * TPS * P : (sl + 1) * TPS * P, :].rearrange(
                        "(i p) h -> p i h", p=P
                    ),
                )
                # keep-warm: a tiny matmul that reads this slab (so it fires
                # as the slab lands) keeps the PE HAM busy-window alive.
                nc.tensor.matmul(
                    warm_ps[:, 0:1], raw[:, 0, 0:128], raw[:, 0, 0:1],
                    start=True, stop=True,
                )
                for i in range(TPS):
                    t = sl * TPS + i
                    prod = scr_pool.tile([P, H], F32, tag="prod")
                    nc.vector.tensor_tensor(
                        out=prod[:], in0=raw[:, i], in1=q_bc[:, b], op=ALU.mult
                    )
                    if t < REDUCE_DVE:
                        nc.vector.tensor_reduce(
                            scores[:, t : t + 1], prod[:], axis=AX.X, op=ALU.add
                        )
                    else:
                        cp = scr_pool.tile([P, H], F32, tag="cp", bufs=1)
                        nc.scalar.activation(
                            cp[:], prod[:], AF.Copy, accum_out=scores[:, t : t + 1]
                        )
                    nc.scalar.copy(enc_bf[:, t], raw[:, i])

            # ---------- softmax over the full 2048 scores ----------
            negm_col = pb_pool.tile([P, 1], F32, tag="negmcol")
            nc.vector.tensor_reduce(
                negm_col[:], scores[:], axis=AX.X, op=ALU.max, negate=True
            )
            e_sb = pb_pool.tile([P, NT], F32, tag="esb")
            s_col = pb_pool.tile([P, 1], F32, tag="scol")
            nc.scalar.activation(
                e_sb[:], scores[:], AF.Exp, bias=negm_col[:], scale=1.0,
                accum_out=s_col[:],
            )
            # bring the 128 per-partition stats onto one partition
            negm_ps = pstp.tile([1, P], F32, tag="tp")
            nc.tensor.transpose(negm_ps[:], negm_col[:], ident_sb[:])
            negm_row = pb_pool.tile([1, P], F32, tag="negmrow")
            nc.vector.tensor_copy(negm_row[:], negm_ps[:])
            s_ps = pstp.tile([1, P], F32, tag="tp")
            nc.tensor.transpose(s_ps[:], s_col[:], ident_sb[:])
            s_row = pb_pool.tile([1, P], F32, tag="srow")
            nc.vector.tensor_copy(s_row[:], s_ps[:])
            # negM = min over partitions of (-rowmax)  (=> M = global max)
            negM = pb_pool.tile([1, 1], F32, tag="negM")
            nc.vector.tensor_reduce(negM[:], negm_row[:], axis=AX.X, op=ALU.min)
            # expm[p] = exp(rowmax_p - M) = exp(-(-rowmax_p) + negM)
            expm = pb_pool.tile([1, P], F32, tag="expm")
            nc.scalar.activation(
                expm[:], negm_row[:], AF.Exp, bias=negM[:], scale=-1.0
            )
            # Z = sum_p rowsum_p * expm_p
            zrow = pb_pool.tile([1, P], F32, tag="zrow")
            nc.vector.tensor_tensor(
                out=zrow[:], in0=s_row[:], in1=expm[:], op=ALU.mult
            )
            zg = pb_pool.tile([1, 1], F32, tag="zg")
            nc.vector.tensor_reduce(zg[:], zrow[:], axis=AX.X, op=ALU.add)
            rz = pb_pool.tile([1, 1], F32, tag="rz")
            nc.vector.reciprocal(rz[:], zg[:])
            prow = pb_pool.tile([1, P], F32, tag="prow")
            nc.vector.tensor_scalar_mul(prow[:], expm[:], rz[:])
            c_ps = pstp.tile([P, 1], F32, tag="tp")
            nc.tensor.transpose(c_ps[:], prow[:], ident_sb[0:1, 0:1])
            c_col = pb_pool.tile([P, 1], F32, tag="ccol")
            nc.vector.tensor_copy(c_col[:], c_ps[:])

            # normalized softmax weights
            w_sb = pb_pool.tile([P, NT], F32, tag="wsb")
            nc.vector.tensor_scalar_mul(w_sb[:], e_sb[:], c_col[:])
            w_bf = pb_pool.tile([P, NT], BF16, tag="wbf")
            nc.vector.tensor_copy(w_bf[:], w_sb[:])

            # attn output: transpose w to [t, p] so DRAM write is contiguous
            wT_ps = pstp.tile([NT, P], F32, tag="tp")
            nc.tensor.transpose(wT_ps[:], w_sb[:], ident_sb[:])
            wT_sb = pb_pool.tile([NT, P], F32, tag="wTsb")
            nc.vector.tensor_copy(wT_sb[:], wT_ps[:])
            nc.gpsimd.dma_start(
                out=out_attn[b].rearrange("(t p) -> t p", p=P), in_=wT_sb[:]
            )

            # ---------- context = sum_s w[s] * enc[s, :]  (bf16) ----------
            ctx_ps = [
                psmm.tile([1, 512], F32, tag="mm", name=f"ctx_ps{i}")
                for i in range(2)
            ]
            for t in range(NT):
                for nh in range(2):
                    nc.tensor.matmul(
                        ctx_ps[nh][:],
                        w_bf[:, t : t + 1],
                        enc_bf[:, t, nh * 512 : (nh + 1) * 512],
                        start=(t == 0),
                        stop=(t == NT - 1),
                    )
            ctx_tanh = pb_pool.tile([1, H], F32, tag="ctxtanh", bufs=1)
            for nh in range(2):
                nc.scalar.activation(
                    ctx_tanh[:, nh * 512 : (nh + 1) * 512], ctx_ps[nh][:], AF.Tanh
                )
            # transpose tanh(ctx) into [j, b] layout (bf16) for final matmul
            for jc in range(HC):
                ct_ps = pstp.tile([P, 1], F32, tag="tp")
                nc.tensor.transpose(
                    ct_ps[:], ctx_tanh[:, jc * P : (jc + 1) * P], ident_sb[0:1, 0:1]
                )
                nc.vector.tensor_copy(tcT_bf[:, jc, b : b + 1], ct_ps[:])

        # ---------- final: relu(tanh([ctx, state]) @ Wc^T + bias) ----------
        f_ps = [
            psmm.tile([BL, 512], F32, tag="mm", name=f"f_ps{i}") for i in range(2)
        ]
        for jc in range(JC):
            lhsT = tcT_bf[:, jc, :] if jc < HC else stT_tanh_bf[:, jc - HC, :]
            for nh in range(2):
                nc.tensor.matmul(
                    f_ps[nh][:],
                    lhsT,
                    wct_sb[:, jc, nh * 512 : (nh + 1) * 512],
                    start=(jc == 0),
                    stop=(jc == JC - 1),
                )
        fout = singles.tile([BL, H], F32)
        for nh in range(2):
            nc.vector.tensor_tensor(
                out=fout[:, nh * 512 : (nh + 1) * 512],
                in0=f_ps[nh][:],
                in1=bias_sb[:, nh * 512 : (nh + 1) * 512],
                op=ALU.add,
            )
        nc.vector.tensor_scalar_max(fout[:], fout[:], 0.0)
        nc.gpsimd.dma_start(out=out_state, in_=fout[:])


def _build(reps=1):
    nc = bacc.Bacc(
        "TRN2", target_bir_lowering=False, debug=False, num_devices=N_CORES
    )
    enc = nc.dram_tensor("enc", [BL, S, H], F32, kind="ExternalInput").ap()
    stateT = nc.dram_tensor("stateT", [H, BL], F32, kind="ExternalInput").ap()
    wa = nc.dram_tensor("wa", [H, H], F32, kind="ExternalInput").ap()
    wct = nc.dram_tensor("wct", [2 * H, H], BF16, kind="ExternalInput").ap()
    bias = nc.dram_tensor("bias", [BL, H], F32, kind="ExternalInput").ap()
    ident = nc.dram_tensor("ident", [P, P], F32, kind="ExternalInput").ap()
    out_state = nc.dram_tensor("out_state", [BL, H], F32, kind="ExternalOutput").ap()
    out_attn = nc.dram_tensor("out_attn", [BL, S], F32, kind="ExternalOutput").ap()

    with tile.TileContext(nc) as tc:
        for _ in range(reps):
            _body(tc, enc, stateT, wa, wct, bias, ident, out_state, out_attn)
    nc.compile()
    return nc


def get_nc(reps=1):
    key = f"nc{reps}"
    if key not in _cache:
        _cache[key] = _build(reps)
    return _cache[key]


def make_in_maps(encoder_output, target_state, Wa_w, Wc_w, Wc_b):
    enc = np.asarray(encoder_output, dtype=np.float32)
    state = np.asarray(target_state, dtype=np.float32)[0]        # [B, H]
    wa = np.ascontiguousarray(np.asarray(Wa_w, dtype=np.float32))
    wct = np.ascontiguousarray(
        np.asarray(Wc_w, dtype=np.float32).T
    ).astype(ml_dtypes.bfloat16)                                 # [2H, H] bf16
    bias = np.tile(np.asarray(Wc_b, dtype=np.float32), (BL, 1))  # [BL, H]
    ident = np.eye(P, dtype=np.float32)

    in_maps = []
    for i in range(N_CORES):
        bs = slice(i * BL, (i + 1) * BL)
        in_maps.append(
            {
                "enc": np.ascontiguousarray(enc[bs]),
                "stateT": np.ascontiguousarray(state[bs].T),
                "wa": wa,
                "wct": wct,
                "bias": bias,
                "ident": ident,
            }
        )
    return in_maps


def kernel(encoder_output, target_state, Wa_w, Wc_w, Wc_b, **run_kwargs):
    nc = get_nc()
    in_maps = make_in_maps(encoder_output, target_state, Wa_w, Wc_w, Wc_b)
    res = run_bass_kernel_spmd(nc, in_maps, core_ids=list(range(N_CORES)), **run_kwargs)
    outs = res.results
    output = np.concatenate([outs[i]["out_state"] for i in range(N_CORES)], axis=0)
    attn = np.concatenate([outs[i]["out_attn"] for i in range(N_CORES)], axis=0)
    kernel.last_results = res
    return output[None].astype(np.float32), attn.astype(np.float32)


# revision 20
# speedup vs baseline: 22.1611x; 1.0098x over previous
"""Luong 'general' attention decode step on 8 TRN2 NeuronCores.

Math (per batch b):
    q[b]      = state[b] @ Wa                      # [H]   (reassociated projection)
    score[s]  = enc[b,s,:] . q[b]                  # [S]
    w         = softmax(score)                     # [S]
    ctx[b]    = sum_s w[s] * enc[b,s,:]            # [H]
    out[b]    = relu(tanh([ctx, state]) @ Wc^T + bias)

Sharding: data-parallel over batch (32 batches -> 4 per core). Weights
replicated. No collectives.

Per-core pipeline:
    - DMA enc in 1MB slabs into a small fp32 ring.
    - Scores in fp32 (attn output precision needs it): DVE tensor_tensor
      multiply against broadcast q, then a free-axis add-reduce; reduces are
      split DVE/ACT to balance the two engines (the dot product costs two
      elementwise passes per tile and the engines co-own them).
    - Each tile is also cast fp32 -> bf16 (ACT) into a per-batch store that
      feeds the context matmul at full PE rate.
    - Softmax: per-partition max/exp/sum, cross-partition combine via
      TensorE transposes, fold exp(m_p - M)/Z into a per-partition scale.
    - Context: TensorE, w column stationary (bf16), enc_bf moving.
    - Epilogue: tanh, 8 chunk transposes to [j, b] layout, final matmul
      against host-pre-transposed Wc^T in bf16, add bias, relu.
    - The PE HAM clock gate idles to 1.2 GHz after ~3.4us without matmuls:
      warmup matmuls at t=0 and one tiny per-slab keep-warm matmul (reads
      the just-DMA'd slab, so it is naturally paced) hold 2.4 GHz.
"""

import numpy as np
import ml_dtypes

import concourse.bass as bass
import concourse.tile as tile
import concourse.mybir as mybir
from concourse import bacc
from concourse.bass_utils import run_bass_kernel_spmd

N_CORES = 8
B, S, H = 32, 2048, 1024
BL = B // N_CORES            # batches per core
P = 128                      # partitions
NT = S // P                  # 16 s-tiles per batch
TPS = 2                      # tiles per DMA slab (1MB slabs)
NSLAB = NT // TPS
HC = H // P                  # 8 h-chunks
JC = 2 * H // P              # 16 j-chunks of the concat axis

F32 = mybir.dt.float32
BF16 = mybir.dt.bfloat16
AF = mybir.ActivationFunctionType
ALU = mybir.AluOpType
AX = mybir.AxisListType

# of the 16 reduces per batch, this many run on DVE (rest on ACT)
REDUCE_DVE = 7
# batch 0 has no cast work on ACT, so fewer reduces go to DVE there
REDUCE_DVE0 = 4

_cache = {}


def _body(tc, enc, stateT, wa, wct, bias, ident, out_state, out_attn):
    nc = tc.nc
    with (
        tc.tile_pool(name="singles", bufs=1) as singles,
        tc.tile_pool(name="encraw", bufs=5) as encraw_pool,
        tc.tile_pool(name="encbf", bufs=2) as encbf_pool,
        tc.tile_pool(name="scr", bufs=3) as scr_pool,
        tc.tile_pool(name="pb", bufs=2) as pb_pool,
        tc.tile_pool(name="psmm", bufs=3, space=bass.MemorySpace.PSUM) as psmm,
        tc.tile_pool(name="psf", bufs=1, space=bass.MemorySpace.PSUM) as psf,
        tc.tile_pool(name="pstp", bufs=2, space=bass.MemorySpace.PSUM) as pstp,
        tc.tile_pool(name="pswarm", bufs=1, space=bass.MemorySpace.PSUM) as pswarm,
    ):
        # ---------- PE warmup: ~4us of matmuls so HAM ungates to 2.4GHz ----------
        warm_sb = singles.tile([P, 512], BF16)
        nc.vector.memset(warm_sb[:], 0.0)
        warm_ps = pswarm.tile([P, 512], F32)
        for i in range(10):
            nc.tensor.matmul(
                warm_ps[:], warm_sb[:, 0:128], warm_sb[:], start=True, stop=True
            )

        # ---------- prologue: constants ----------
        # Small gpsimd loads first so the q bounce DMAs are not queued behind
        # a multi-MB transfer; wct comes after the q broadcast.
        ident_sb = singles.tile([P, P], F32)
        nc.gpsimd.dma_start(out=ident_sb[:], in_=ident)
        stT_sb = singles.tile([P, HC, BL], F32)
        nc.gpsimd.dma_start(
            out=stT_sb[:], in_=stateT.rearrange("(c p) b -> p c b", p=P)
        )
        bias_sb = singles.tile([BL, H], F32)
        nc.gpsimd.dma_start(out=bias_sb[:], in_=bias)

        # Wa arrives in 8 chunk DMAs so the q matmuls pipeline with the load.
        wa_sb = singles.tile([P, HC, H], F32, tag="wgt")
        wa_r = wa.rearrange("(c p) h -> p c h", p=P)
        for c in range(HC):
            nc.sync.dma_start(out=wa_sb[:, c], in_=wa_r[:, c])

        # tanh(state)^T in bf16 (state half of the final concat matmul)
        stT_tanh_bf = singles.tile([P, HC, BL], BF16)
        nc.scalar.activation(stT_tanh_bf[:], stT_sb[:], AF.Tanh)

        # ---------- q = state @ Wa  -> [BL, H] ----------
        q_ps = [
            psmm.tile([BL, 512], F32, tag="mm", name=f"q_ps{i}") for i in range(2)
        ]
        for c in range(HC):
            for nh in range(2):
                nc.tensor.matmul(
                    q_ps[nh][:],
                    stT_sb[:, c, :],
                    wa_sb[:, c, nh * 512 : (nh + 1) * 512],
                    start=(c == 0),
                    stop=(c == HC - 1),
                )
        q_sb = singles.tile([BL, H], F32)
        for nh in range(2):
            nc.scalar.copy(q_sb[:, nh * 512 : (nh + 1) * 512], q_ps[nh][:])

        # ---------- broadcast q[b] to all 128 partitions ----------
        # DRAM bounce + partition-stride-0 read replicates q across partitions.
        q_dram = nc.dram_tensor(f"q_dram_{nc.next_id()}", [BL, H], F32).ap()
        nc.gpsimd.dma_start(out=q_dram, in_=q_sb[:])
        q_bc = singles.tile([P, BL, H], F32)
        q_rep = bass.AP(
            tensor=q_dram.tensor,
            offset=q_dram.offset,
            ap=[[0, P], q_dram.ap[0], q_dram.ap[1]],
        )
        nc.gpsimd.dma_start(out=q_bc[:], in_=q_rep)

        # Wc^T (bf16) reuses Wa's SBUF slot once q is done (same tag)
        wct_sb = singles.tile([P, JC, H], BF16, tag="wgt")
        nc.gpsimd.dma_start(out=wct_sb[:], in_=wct.rearrange("(c p) o -> p c o", p=P))

        # tanh(ctx)^T assembled across batches for the final matmul
        tcT_bf = singles.tile([P, HC, BL], BF16)

        # Final-output PSUM accumulates across the whole kernel: the state
        # half of the concat matmul runs in the prologue, the ctx half lands
        # per-batch as each context finishes.
        f_ps = [
            psf.tile([BL, 512], F32, tag="fps", name=f"f_ps{i}") for i in range(2)
        ]
        for jc in range(HC):
            for nh in range(2):
                nc.tensor.matmul(
                    f_ps[nh][:],
                    stT_tanh_bf[:, jc, :],
                    wct_sb[:, HC + jc, nh * 512 : (nh + 1) * 512],
                    start=(jc == 0),
                    stop=False,
                )

        # ---------- streaming main loop ----------
        for b in range(BL):
            scores = pb_pool.tile([P, NT], F32, tag="scores")
            enc_bf = encbf_pool.tile([P, NT, H], BF16, tag="encbf")
            rd = REDUCE_DVE

            for sl in range(NSLAB):
                raw = encraw_pool.tile(
                    [P, TPS, H], F32, tag="raw", name="raw"
                )[:]
                eng = nc.sync if sl % 2 == 0 else nc.gpsimd
                eng.dma_start(
                    out=raw,
                    in_=enc[b, sl * TPS * P : (sl + 1) * TPS * P, :].rearrange(
                        "(i p) h -> p i h", p=P
                    ),
                )
                # keep-warm: a tiny matmul that reads this slab (so it fires
                # as the slab lands) keeps the PE HAM busy-window alive.
                nc.tensor.matmul(
                    warm_ps[:, 0:1], raw[:, 0, 0:128], raw[:, 0, 0:1],
                    start=True, stop=True,
                )
                for i in range(TPS):
                    t = sl * TPS + i
                    prod = scr_pool.tile([P, H], F32, tag="prod")
                    nc.vector.tensor_tensor(
                        out=prod[:], in0=raw[:, i], in1=q_bc[:, b], op=ALU.mult
                    )
                    if t % NT < rd:
                        nc.vector.tensor_reduce(
                            scores[:, t : t + 1], prod[:], axis=AX.X, op=ALU.add
                        )
                    else:
                        cp = scr_pool.tile([P, H], F32, tag="prod", name="cp")
                        nc.scalar.activation(
                            cp[:], prod[:], AF.Copy, accum_out=scores[:, t : t + 1]
                        )
                    nc.scalar.copy(enc_bf[:, t], raw[:, i])

            # ---------- softmax over the full 2048 scores ----------
            negm_col = pb_pool.tile([P, 1], F32, tag="negmcol")
            nc.vector.tensor_reduce(
                negm_col[:], scores[:], axis=AX.X, op=ALU.max, negate=True
            )
            e_sb = pb_pool.tile([P, NT], F32, tag="esb")
            s_col = pb_pool.tile([P, 1], F32, tag="scol")
            nc.scalar.activation(
                e_sb[:], scores[:], AF.Exp, bias=negm_col[:], scale=1.0,
                accum_out=s_col[:],
            )
            # bring the 128 per-partition stats onto one partition
            negm_ps = pstp.tile([1, P], F32, tag="tp")
            nc.tensor.transpose(negm_ps[:], negm_col[:], ident_sb[:])
            negm_row = pb_pool.tile([1, P], F32, tag="negmrow")
            nc.vector.tensor_copy(negm_row[:], negm_ps[:])
            s_ps = pstp.tile([1, P], F32, tag="tp")
            nc.tensor.transpose(s_ps[:], s_col[:], ident_sb[:])
            s_row = pb_pool.tile([1, P], F32, tag="srow")
            nc.vector.tensor_copy(s_row[:], s_ps[:])
            # negM = min over partitions of (-rowmax)  (=> M = global max)
            negM = pb_pool.tile([1, 1], F32, tag="negM")
            nc.vector.tensor_reduce(negM[:], negm_row[:], axis=AX.X, op=ALU.min)
            # expm[p] = exp(rowmax_p - M) = exp(-(-rowmax_p) + negM)
            expm = pb_pool.tile([1, P], F32, tag="expm")
            nc.scalar.activation(
                expm[:], negm_row[:], AF.Exp, bias=negM[:], scale=-1.0
            )
            # Z = sum_p rowsum_p * expm_p
            zrow = pb_pool.tile([1, P], F32, tag="zrow")
            nc.vector.tensor_tensor(
                out=zrow[:], in0=s_row[:], in1=expm[:], op=ALU.mult
            )
            zg = pb_pool.tile([1, 1], F32, tag="zg")
            nc.vector.tensor_reduce(zg[:], zrow[:], axis=AX.X, op=ALU.add)
            rz = pb_pool.tile([1, 1], F32, tag="rz")
            nc.vector.reciprocal(rz[:], zg[:])
            prow = pb_pool.tile([1, P], F32, tag="prow")
            nc.vector.tensor_scalar_mul(prow[:], expm[:], rz[:])
            c_ps = pstp.tile([P, 1], F32, tag="tp")
            nc.tensor.transpose(c_ps[:], prow[:], ident_sb[0:1, 0:1])
            c_col = pb_pool.tile([P, 1], F32, tag="ccol")
            nc.vector.tensor_copy(c_col[:], c_ps[:])

            # normalized softmax weights
            w_sb = pb_pool.tile([P, NT], F32, tag="wsb")
            nc.vector.tensor_scalar_mul(w_sb[:], e_sb[:], c_col[:])
            w_bf = pb_pool.tile([P, NT], BF16, tag="wbf")
            nc.vector.tensor_copy(w_bf[:], w_sb[:])

            # attn output: transpose w to [t, p] so DRAM write is contiguous
            wT_ps = pstp.tile([NT, P], F32, tag="tp")
            nc.tensor.transpose(wT_ps[:], w_sb[:], ident_sb[:])
            wT_sb = pb_pool.tile([NT, P], F32, tag="wTsb")
            nc.vector.tensor_copy(wT_sb[:], wT_ps[:])
            nc.gpsimd.dma_start(
                out=out_attn[b].rearrange("(t p) -> t p", p=P), in_=wT_sb[:]
            )

            # ---------- context = sum_s w[s] * enc[s, :] ----------
            ctx_ps = [
                psmm.tile([1, 512], F32, tag="mm", name=f"ctx_ps{i}")
                for i in range(2)
            ]
            for t in range(NT):
                for nh in range(2):
                    nc.tensor.matmul(
                        ctx_ps[nh][:],
                        w_bf[:, t : t + 1],
                        enc_bf[:, t, nh * 512 : (nh + 1) * 512],
                        start=(t == 0),
                        stop=(t == NT - 1),
                    )
            ctx_tanh = pb_pool.tile([1, H], F32, tag="ctxtanh", bufs=1)
            for nh in range(2):
                nc.scalar.activation(
                    ctx_tanh[:, nh * 512 : (nh + 1) * 512], ctx_ps[nh][:], AF.Tanh
                )
            # transpose tanh(ctx) into [j, b] layout (bf16) for final matmul
            for jc in range(HC):
                ct_ps = pstp.tile([P, 1], F32, tag="tp")
                nc.tensor.transpose(
                    ct_ps[:], ctx_tanh[:, jc * P : (jc + 1) * P], ident_sb[0:1, 0:1]
                )
                nc.vector.tensor_copy(tcT_bf[:, jc, b : b + 1], ct_ps[:])
        # ctx half of the final matmul (after all batches' tcT assembled)
        for jc in range(HC):
            for nh in range(2):
                nc.tensor.matmul(
                    f_ps[nh][:],
                    tcT_bf[:, jc, :],
                    wct_sb[:, jc, nh * 512 : (nh + 1) * 512],
                    start=False,
                    stop=(jc == HC - 1),
                )
        fout = singles.tile([BL, H], F32)
        for nh in range(2):
            nc.vector.tensor_tensor(
                out=fout[:, nh * 512 : (nh + 1) * 512],
                in0=f_ps[nh][:],
                in1=bias_sb[:, nh * 512 : (nh + 1) * 512],
                op=ALU.add,
            )
        nc.vector.tensor_scalar_max(fout[:], fout[:], 0.0)
        nc.gpsimd.dma_start(out=out_state, in_=fout[:])


def _build(reps=1):
    nc = bacc.Bacc(
        "TRN2", target_bir_lowering=False, debug=False, num_devices=N_CORES
    )
    enc = nc.dram_tensor("enc", [BL, S, H], F32, kind="ExternalInput").ap()
    stateT = nc.dram_tensor("stateT", [H, BL], F32, kind="ExternalInput").ap()
    wa = nc.dram_tensor("wa", [H, H], F32, kind="ExternalInput").ap()
    wct = nc.dram_tensor("wct", [2 * H, H], BF16, kind="ExternalInput").ap()
    bias = nc.dram_tensor("bias", [BL, H], F32, kind="ExternalInput").ap()
    ident = nc.dram_tensor("ident", [P, P], F32, kind="ExternalInput").ap()
    out_state = nc.dram_tensor("out_state", [BL, H], F32, kind="ExternalOutput").ap()
    out_attn = nc.dram_tensor("out_attn", [BL, S], F32, kind="ExternalOutput").ap()

    with tile.TileContext(nc) as tc:
        for _ in range(reps):
            _body(tc, enc, stateT, wa, wct, bias, ident, out_state, out_attn)
    nc.compile()
    return nc


def get_nc(reps=1):
    key = f"nc{reps}"
    if key not in _cache:
        _cache[key] = _build(reps)
    return _cache[key]


def make_in_maps(encoder_output, target_state, Wa_w, Wc_w, Wc_b):
    enc = np.asarray(encoder_output, dtype=np.float32)
    state = np.asarray(target_state, dtype=np.float32)[0]        # [B, H]
    wa = np.ascontiguousarray(np.asarray(Wa_w, dtype=np.float32))
    wct = np.ascontiguousarray(
        np.asarray(Wc_w, dtype=np.float32).T
    ).astype(ml_dtypes.bfloat16)                                 # [2H, H] bf16
    bias = np.tile(np.asarray(Wc_b, dtype=np.float32), (BL, 1))  # [BL, H]
    ident = np.eye(P, dtype=np.float32)

    in_maps = []
    for i in range(N_CORES):
        bs = slice(i * BL, (i + 1) * BL)
        in_maps.append(
            {
                "enc": np.ascontiguousarray(enc[bs]),
                "stateT": np.ascontiguousarray(state[bs].T),
                "wa": wa,
                "wct": wct,
                "bias": bias,
                "ident": ident,
            }
        )
    return in_maps


def kernel(encoder_output, target_state, Wa_w, Wc_w, Wc_b, **run_kwargs):
    nc = get_nc()
    in_maps = make_in_maps(encoder_output, target_state, Wa_w, Wc_w, Wc_b)
    res = run_bass_kernel_spmd(nc, in_maps, core_ids=list(range(N_CORES)), **run_kwargs)
    outs = res.results
    output = np.concatenate([outs[i]["out_state"] for i in range(N_CORES)], axis=0)
    attn = np.concatenate([outs[i]["out_attn"] for i in range(N_CORES)], axis=0)
    kernel.last_results = res
    return output[None].astype(np.float32), attn.astype(np.float32)


# revision 22
# speedup vs baseline: 23.8071x; 1.0743x over previous
"""Luong 'general' attention decode step on 8 TRN2 NeuronCores.

Math (per batch b):
    q[b]      = state[b] @ Wa                      # [H]   (reassociated projection)
    score[s]  = enc[b,s,:] . q[b]                  # [S]
    w         = softmax(score)                     # [S]
    ctx[b]    = sum_s w[s] * enc[b,s,:]            # [H]
    out[b]    = relu(tanh([ctx, state]) @ Wc^T + bias)

Sharding: data-parallel over batch (32 batches -> 4 per core). Weights
replicated. No collectives.

Per-core pipeline (two elementwise passes per enc tile total):
    - DMA enc in 1MB slabs into a small fp32 ring (slabs alternate between
      the sync and gpsimd DMA queues).
    - DVE computes prod = enc * q_bcast (fp32).
    - ACT runs ONE fused op per tile: out = bf16(prod) into a per-batch
      store, accum_out = sum_h prod = the score column (fp32 accumulate).
    - Context reuses the bf16 prod store: sum_s w_s*prod[s,h] = q_h*ctx[h],
      so ctx = (TensorE bf16 matmul over prod) * 1/q elementwise. The bf16
      rounding of prod scales with q_h and divides back out - no precision
      amplification, and scores stay exact fp32.
    - q = state @ Wa via a bf16 hi/lo split (3 matmuls, error ~2^-16),
      pipelined against the chunked Wa DMA; broadcast across partitions via
      a DRAM bounce on the ACT HWDGE queue, per-batch slices.
    - Softmax per batch: per-partition max/exp/sum, cross-partition combine
      via TensorE transposes; emitted in 6 stages interleaved into the NEXT
      batch's streaming so the serial cross-engine chain hides.
    - Final: state half of the concat matmul accumulates into PSUM early;
      ctx half lands after the last batch; add bias, relu.
    - PE HAM warmup matmuls at t=0 + per-slab keep-warm matmuls hold the
      2.4GHz clock.
"""

import numpy as np
import ml_dtypes

import concourse.bass as bass
import concourse.tile as tile
import concourse.mybir as mybir
from concourse import bacc
from concourse.bass_utils import run_bass_kernel_spmd

N_CORES = 8
B, S, H = 32, 2048, 1024
BL = B // N_CORES            # batches per core
P = 128                      # partitions
NT = S // P                  # 16 s-tiles per batch
TPS = 2                      # tiles per DMA slab (1MB slabs)
NSLAB = NT // TPS
HC = H // P                  # 8 h-chunks
JC = 2 * H // P              # 16 j-chunks of the concat axis

F32 = mybir.dt.float32
BF16 = mybir.dt.bfloat16
AF = mybir.ActivationFunctionType
ALU = mybir.AluOpType
AX = mybir.AxisListType

_cache = {}


def _body(tc, enc, stateT, wa_hl, wct, bias, ident, out_state, out_attn):
    nc = tc.nc
    with (
        tc.tile_pool(name="singles", bufs=1) as singles,
        tc.tile_pool(name="encraw", bufs=4) as encraw_pool,
        tc.tile_pool(name="pbf", bufs=2) as pbf_pool,
        tc.tile_pool(name="scr", bufs=3) as scr_pool,
        tc.tile_pool(name="pb", bufs=2) as pb_pool,
        tc.tile_pool(name="psmm", bufs=3, space=bass.MemorySpace.PSUM) as psmm,
        tc.tile_pool(name="psf", bufs=1, space=bass.MemorySpace.PSUM) as psf,
        tc.tile_pool(name="pstp", bufs=2, space=bass.MemorySpace.PSUM) as pstp,
        tc.tile_pool(name="pswarm", bufs=1, space=bass.MemorySpace.PSUM) as pswarm,
    ):
        # ---------- PE warmup: ~4us of matmuls so HAM ungates to 2.4GHz ----------
        warm_sb = singles.tile([P, 512], BF16)
        nc.vector.memset(warm_sb[:], 0.0)
        warm_ps = pswarm.tile([P, 512], F32)
        for i in range(10):
            nc.tensor.matmul(
                warm_ps[:], warm_sb[:, 0:128], warm_sb[:], start=True, stop=True
            )

        # ---------- prologue: constants ----------
        ident_sb = singles.tile([P, P], F32)
        nc.gpsimd.dma_start(out=ident_sb[:], in_=ident)
        stT_sb = singles.tile([P, HC, BL], F32)
        nc.gpsimd.dma_start(
            out=stT_sb[:], in_=stateT.rearrange("(c p) b -> p c b", p=P)
        )
        bias_sb = singles.tile([BL, H], F32)
        nc.gpsimd.dma_start(out=bias_sb[:], in_=bias)

        # Wa (bf16 hi+lo planes) in per-chunk DMAs; q matmuls pipeline behind.
        wa_sb = singles.tile([P, 2, HC, H], BF16, tag="wgt")
        wa_r = wa_hl.rearrange("t (c p) h -> p t c h", p=P)
        for c in range(HC):
            for tpl in range(2):
                nc.sync.dma_start(out=wa_sb[:, tpl, c], in_=wa_r[:, tpl, c])

        # state^T bf16 hi/lo planes (from fp32 on-chip; cheap)
        stT_hl = singles.tile([P, 2, HC, BL], BF16)
        nc.vector.tensor_copy(stT_hl[:, 0], stT_sb[:])              # hi
        stT_lo32 = singles.tile([P, HC, BL], F32)
        nc.vector.tensor_tensor(
            out=stT_lo32[:], in0=stT_sb[:], in1=stT_hl[:, 0], op=ALU.subtract
        )
        nc.vector.tensor_copy(stT_hl[:, 1], stT_lo32[:])            # lo

        # tanh(state)^T in bf16 (state half of the final concat matmul)
        stT_tanh_bf = singles.tile([P, HC, BL], BF16)
        nc.scalar.activation(stT_tanh_bf[:], stT_sb[:], AF.Tanh)

        # ---------- q = state @ Wa (bf16 hi/lo, 3 cross terms) ----------
        q_ps = [
            psmm.tile([BL, 512], F32, tag="mm", name=f"q_ps{i}") for i in range(2)
        ]
        TERMS = [(0, 0), (1, 0), (0, 1)]   # (state plane, Wa plane)
        for c in range(HC):
            for nh in range(2):
                for ti, (sp, wp) in enumerate(TERMS):
                    nc.tensor.matmul(
                        q_ps[nh][:],
                        stT_hl[:, sp, c, :],
                        wa_sb[:, wp, c, nh * 512 : (nh + 1) * 512],
                        start=(c == 0 and ti == 0),
                        stop=(c == HC - 1 and ti == len(TERMS) - 1),
                    )
        q_sb = singles.tile([BL, H], F32)
        for nh in range(2):
            nc.scalar.copy(q_sb[:, nh * 512 : (nh + 1) * 512], q_ps[nh][:])

        # ---------- broadcast q[b] to all 128 partitions ----------
        # DRAM bounce + partition-stride-0 read, on the ACT HWDGE queue so it
        # is not stuck behind bulk transfers; per-batch slices so batch 0's
        # multiplies ungate as soon as its slice lands.
        q_dram = nc.dram_tensor(f"q_dram_{nc.next_id()}", [BL, H], F32).ap()
        nc.scalar.dma_start(out=q_dram, in_=q_sb[:])
        q_bc = singles.tile([P, BL, H], F32)
        for b in range(BL):
            q_rep = bass.AP(
                tensor=q_dram.tensor,
                offset=q_dram.offset + b * H,
                ap=[[0, P], [1, H]],
            )
            nc.scalar.dma_start(out=q_bc[:, b], in_=q_rep)

        # Wc^T (bf16) reuses Wa's SBUF slot once q is done (same tag);
        # emitted later on gpsimd so batch 0/1 slabs go first on that queue.
        # tanh(ctx)^T assembled across batches for the final matmul
        tcT_bf = singles.tile([P, HC, BL], BF16)

        # ---------- per-batch softmax/context stages ----------
        def s1(b, st):
            nc.vector.tensor_reduce(
                st["negm"][:], st["scores"][:], axis=AX.X, op=ALU.max, negate=True
            )
            nc.scalar.activation(
                st["e"][:], st["scores"][:], AF.Exp, bias=st["negm"][:],
                scale=1.0, accum_out=st["scol"][:],
            )

        def s2(b, st):
            negm_ps = pstp.tile([1, P], F32, tag="tp", name="negm_ps")
            nc.tensor.transpose(negm_ps[:], st["negm"][:], ident_sb[:])
            nc.vector.tensor_copy(st["negmrow"][:], negm_ps[:])
            s_ps = pstp.tile([1, P], F32, tag="tp", name="s_ps")
            nc.tensor.transpose(s_ps[:], st["scol"][:], ident_sb[:])
            nc.vector.tensor_copy(st["srow"][:], s_ps[:])
            nc.vector.tensor_reduce(
                st["negM"][:], st["negmrow"][:], axis=AX.X, op=ALU.min
            )
            nc.scalar.activation(
                st["expm"][:], st["negmrow"][:], AF.Exp, bias=st["negM"][:],
                scale=-1.0,
            )

        def s3(b, st):
            nc.vector.tensor_tensor(
                out=st["zrow"][:], in0=st["srow"][:], in1=st["expm"][:], op=ALU.mult
            )
            nc.vector.tensor_reduce(st["zg"][:], st["zrow"][:], axis=AX.X, op=ALU.add)
            nc.vector.reciprocal(st["rz"][:], st["zg"][:])
            nc.vector.tensor_scalar_mul(st["prow"][:], st["expm"][:], st["rz"][:])
            c_ps = pstp.tile([P, 1], F32, tag="tp", name="c_ps")
            nc.tensor.transpose(c_ps[:], st["prow"][:], ident_sb[0:1, 0:1])
            nc.vector.tensor_copy(st["ccol"][:], c_ps[:])

        def s4(b, st):
            nc.vector.tensor_scalar_mul(st["w"][:], st["e"][:], st["ccol"][:])
            nc.vector.tensor_copy(st["wbf"][:], st["w"][:])
            wT_ps = pstp.tile([NT, P], F32, tag="tp", name="wT_ps")
            nc.tensor.transpose(wT_ps[:], st["w"][:], ident_sb[:])
            wT_sb = pb_pool.tile([NT, P], F32, tag="wTsb", name="wT_sb")
            nc.vector.tensor_copy(wT_sb[:], wT_ps[:])
            nc.gpsimd.dma_start(
                out=out_attn[b].rearrange("(t p) -> t p", p=P), in_=wT_sb[:]
            )
            # 1/q row (partition 0 copy lives in q_bc) for the ctx rescale
            nc.vector.reciprocal(st["rq"][:], q_bc[0:1, b, :])

        def s5(b, st):
            ctx_ps = [
                psmm.tile([1, 512], F32, tag="mm", name=f"ctx_ps{i}")
                for i in range(2)
            ]
            st["ctx_ps"] = ctx_ps
            for t in range(NT):
                for nh in range(2):
                    nc.tensor.matmul(
                        ctx_ps[nh][:],
                        st["wbf"][:, t : t + 1],
                        st["pbf"][:, t, nh * 512 : (nh + 1) * 512],
                        start=(t == 0),
                        stop=(t == NT - 1),
                    )

        def s6(b, st):
            # ctx = (sum_s w*prod) / q ; then tanh; then to [j, b] bf16 layout
            ctxn = pb_pool.tile([1, H], F32, tag="ctxn", bufs=1, name="ctxn")
            for nh in range(2):
                nc.vector.tensor_tensor(
                    out=ctxn[:, nh * 512 : (nh + 1) * 512],
                    in0=st["ctx_ps"][nh][:],
                    in1=st["rq"][:, nh * 512 : (nh + 1) * 512],
                    op=ALU.mult,
                )
            ctx_tanh = pb_pool.tile([1, H], F32, tag="ctxtanh", bufs=1,
                                    name="ctx_tanh")
            nc.scalar.activation(ctx_tanh[:], ctxn[:], AF.Tanh)
            for jc in range(HC):
                ct_ps = pstp.tile([P, 1], F32, tag="tp", name="ct_ps")
                nc.tensor.transpose(
                    ct_ps[:], ctx_tanh[:, jc * P : (jc + 1) * P], ident_sb[0:1, 0:1]
                )
                nc.vector.tensor_copy(tcT_bf[:, jc, b : b + 1], ct_ps[:])

        stages = [s1, s2, s3, s4, s5, s6]

        def make_state(b):
            return {
                "scores": pb_pool.tile([P, NT], F32, tag="scores", name="scores"),
                "pbf": pbf_pool.tile([P, NT, H], BF16, tag="pbf", name="pbf"),
                "negm": pb_pool.tile([P, 1], F32, tag="negm", name="negm"),
                "e": pb_pool.tile([P, NT], F32, tag="e", name="e"),
                "scol": pb_pool.tile([P, 1], F32, tag="scol", name="scol"),
                "negmrow": pb_pool.tile([1, P], F32, tag="negmrow", name="negmrow"),
                "srow": pb_pool.tile([1, P], F32, tag="srow", name="srow"),
                "negM": pb_pool.tile([1, 1], F32, tag="negM", name="negM"),
                "expm": pb_pool.tile([1, P], F32, tag="expm", name="expm"),
                "zrow": pb_pool.tile([1, P], F32, tag="zrow", name="zrow"),
                "zg": pb_pool.tile([1, 1], F32, tag="zg", name="zg"),
                "rz": pb_pool.tile([1, 1], F32, tag="rz", name="rz"),
                "prow": pb_pool.tile([1, P], F32, tag="prow", name="prow"),
                "ccol": pb_pool.tile([P, 1], F32, tag="ccol", name="ccol"),
                "w": pb_pool.tile([P, NT], F32, tag="w", name="w"),
                "wbf": pb_pool.tile([P, NT], BF16, tag="wbf", name="wbf"),
                "rq": pb_pool.tile([1, H], F32, tag="rq", name="rq"),
            }

        # ---------- streaming main loop with interleaved softmax stages ----------
        prev = None   # (b, state) whose softmax stages are pending
        cur = None
        for b in range(BL):
            cur = make_state(b)
            for sl in range(NSLAB):
                raw = encraw_pool.tile(
                    [P, TPS, H], F32, tag="raw", name="raw"
                )[:]
                eng = nc.gpsimd if sl % 2 == 0 else nc.sync
                eng.dma_start(
                    out=raw,
                    in_=enc[b, sl * TPS * P : (sl + 1) * TPS * P, :].rearrange(
                        "(i p) h -> p i h", p=P
                    ),
                )
                # keep-warm matmul paced by this slab's arrival
                nc.tensor.matmul(
                    warm_ps[:, 0:1], raw[:, 0, 0:128], raw[:, 0, 0:1],
                    start=True, stop=True,
                )
                for i in range(TPS):
                    t = sl * TPS + i
                    prod = scr_pool.tile([P, H], F32, tag="prod", name="prod")
                    nc.vector.tensor_tensor(
                        out=prod[:], in0=raw[:, i], in1=q_bc[:, b], op=ALU.mult
                    )
                    # fused: bf16 store of prod + fp32 row-sum = score column
                    nc.scalar.activation(
                        cur["pbf"][:, t], prod[:], AF.Copy,
                        accum_out=cur["scores"][:, t : t + 1],
                    )
                # previous batch's softmax trickles through, one stage per slab
                if prev is not None and 1 <= sl <= len(stages):
                    stages[sl - 1](prev[0], prev[1])
                    if sl == len(stages):
                        prev = None
            if b == 0:
                # bulk Wc^T load + state half of the final matmul, placed here
                # so batch 0/1 enc slabs lead the gpsimd queue
                wct_sb = singles.tile([P, JC, H], BF16, tag="wgt")
                nc.gpsimd.dma_start(
                    out=wct_sb[:], in_=wct.rearrange("(c p) o -> p c o", p=P)
                )
                f_ps = [
                    psf.tile([BL, 512], F32, tag="fps", name=f"f_ps{i}")
                    for i in range(2)
                ]
                for jc in range(HC):
                    for nh in range(2):
                        nc.tensor.matmul(
                            f_ps[nh][:],
                            stT_tanh_bf[:, jc, :],
                            wct_sb[:, HC + jc, nh * 512 : (nh + 1) * 512],
                            start=(jc == 0),
                            stop=False,
                        )
            prev = (b, cur)

        # drain the last batch's stages
        for fn in stages:
            fn(prev[0], prev[1])

        # ---------- final: relu(tanh([ctx, state]) @ Wc^T + bias) ----------
        for jc in range(HC):
            for nh in range(2):
                nc.tensor.matmul(
                    f_ps[nh][:],
                    tcT_bf[:, jc, :],
                    wct_sb[:, jc, nh * 512 : (nh + 1) * 512],
                    start=False,
                    stop=(jc == HC - 1),
                )
        fout = singles.tile([BL, H], F32)
        for nh in range(2):
            nc.vector.tensor_tensor(
                out=fout[:, nh * 512 : (nh + 1) * 512],
                in0=f_ps[nh][:],
                in1=bias_sb[:, nh * 512 : (nh + 1) * 512],
                op=ALU.add,
            )
        nc.vector.tensor_scalar_max(fout[:], fout[:], 0.0)
        nc.gpsimd.dma_start(out=out_state, in_=fout[:])


def _build(reps=1):
    nc = bacc.Bacc(
        "TRN2", target_bir_lowering=False, debug=False, num_devices=N_CORES
    )
    enc = nc.dram_tensor("enc", [BL, S, H], F32, kind="ExternalInput").ap()
    stateT = nc.dram_tensor("stateT", [H, BL], F32, kind="ExternalInput").ap()
    wa_hl = nc.dram_tensor("wa_hl", [2, H, H], BF16, kind="ExternalInput").ap()
    wct = nc.dram_tensor("wct", [2 * H, H], BF16, kind="ExternalInput").ap()
    bias = nc.dram_tensor("bias", [BL, H], F32, kind="ExternalInput").ap()
    ident = nc.dram_tensor("ident", [P, P], F32, kind="ExternalInput").ap()
    out_state = nc.dram_tensor("out_state", [BL, H], F32, kind="ExternalOutput").ap()
    out_attn = nc.dram_tensor("out_attn", [BL, S], F32, kind="ExternalOutput").ap()

    with tile.TileContext(nc) as tc:
        for _ in range(reps):
            _body(tc, enc, stateT, wa_hl, wct, bias, ident, out_state, out_attn)
    nc.compile()
    return nc


def get_nc(reps=1):
    key = f"nc{reps}"
    if key not in _cache:
        _cache[key] = _build(reps)
    return _cache[key]


def make_in_maps(encoder_output, target_state, Wa_w, Wc_w, Wc_b):
    enc = np.asarray(encoder_output, dtype=np.float32)
    state = np.asarray(target_state, dtype=np.float32)[0]        # [B, H]
    wa = np.ascontiguousarray(np.asarray(Wa_w, dtype=np.float32))
    wa_hi = wa.astype(ml_dtypes.bfloat16)
    wa_lo = (wa - wa_hi.astype(np.float32)).astype(ml_dtypes.bfloat16)
    wa_hl = np.ascontiguousarray(np.stack([wa_hi, wa_lo]))       # [2, H, H]
    wct = np.ascontiguousarray(
        np.asarray(Wc_w, dtype=np.float32).T
    ).astype(ml_dtypes.bfloat16)                                 # [2H, H] bf16
    bias = np.tile(np.asarray(Wc_b, dtype=np.float32), (BL, 1))  # [BL, H]
    ident = np.eye(P, dtype=np.float32)

    in_maps = []
    for i in range(N_CORES):
        bs = slice(i * BL, (i + 1) * BL)
        in_maps.append(
            {
                "enc": np.ascontiguousarray(enc[bs]),
                "stateT": np.ascontiguousarray(state[bs].T),
                "wa_hl": wa_hl,
                "wct": wct,
                "bias": bias,
                "ident": ident,
            }
        )
    return in_maps


def kernel(encoder_output, target_state, Wa_w, Wc_w, Wc_b, **run_kwargs):
    nc = get_nc()
    in_maps = make_in_maps(encoder_output, target_state, Wa_w, Wc_w, Wc_b)
    res = run_bass_kernel_spmd(nc, in_maps, core_ids=list(range(N_CORES)), **run_kwargs)
    outs = res.results
    output = np.concatenate([outs[i]["out_state"] for i in range(N_CORES)], axis=0)
    attn = np.concatenate([outs[i]["out_attn"] for i in range(N_CORES)], axis=0)
    kernel.last_results = res
    return output[None].astype(np.float32), attn.astype(np.float32)
